# revision 27
# baseline (speedup 1.0000x reference)
"""Trainium2 Bass kernel for a single DeBERTa-style attention head.

Problem shapes (hardcoded):
  B=8, S=2048, E=768(n_embed), H=64(head)
  q = I @ Wq + bq ; k = x @ Wk + bk ; v = x @ Wv + bv
  w = (q @ k^T) / sqrt(E) ; w = where(mask==0, -1e9, w)
  scores = softmax(w, axis=-1) ; out = scores @ v

Sharding: data-parallel over batch B across the 8 NeuronCores (one batch
element per core, identical SPMD program).

Bias algebra (exact, removes every device-side bias op):
  * bk contributes q.bk to w[q, :] - constant over k, softmax-invariant:
    dropped.
  * bv contributes (sum_k scores)*bv = bv: added on the host.
  * bq contributes d[k] = x[k].(Wk bq); exp factorizes, so v_aug rows are
    scaled by ed = exp(d/sqrt(E)). d rides as a 65th column of the v
    projection (host packs [Wv | Wk@bq]); the 65th v_aug column is ed, which
    also yields the softmax denominator through the ctx matmul.

Per-core dataflow (bf16 operands, fp32 PSUM):
  1. Host pre-bands IT/XT so every 512-q band is one per-partition-contiguous
     768KB HWDGE DMA (sequential HBM reads, 6KB descriptors).
  2. qT/kT computed twice via concurrent column-tiled matmul chains so copies
     land on partitions 0:64 AND 64:128 (DVE cannot cross partitions; the
     duplicate chain uses otherwise-idle PE column groups).
  3. qk runs 2-way row-tiled: K=64 only uses half the PE rows, so ki-even
     (partitions 0:64) and ki-odd (partitions 64:128) matmul concurrently.
  4. Middle loop over (qh, ki-pair, qq): wp [128,1024] f32 (2 banks, x2
     ping-pong) -> one ACT exp call (PSUM->SBUF, the ~32us floor) -> the
     uint8 mask chunk is applied by a SWDGE cast-DMA with accum_op=mult
     straight into the exp output (host pre-reorders mask into per-iteration
     blocks) -> 2 ctx matmuls accumulate ctxT [65,1024] (lhsT = v_aug).
  5. DVE copies ctxT -> SBUF, DMA out ctx^T+denominator; host divides,
     transposes, adds bv.
"""

import math
from contextlib import ExitStack

import numpy as np

import concourse.bass as bass
import concourse.tile as tile
import concourse.mybir as mybir
from concourse import bacc
from concourse.bass_utils import run_bass_kernel_spmd

B, S, E, H = 8, 2048, 768, 64
N_CORES = 8
SC = S // 128   # 16 seq chunks (ki)
EC = E // 128   # 6 embed chunks
SCALE = 1.0 / math.sqrt(E)
NITER = 32      # 2 qh x 8 pairs x 2 qq

F32 = mybir.dt.float32
BF16 = mybir.dt.bfloat16
AF = mybir.ActivationFunctionType
ALU = mybir.AluOpType

_cache = {}


def _build_program():
    nc = bacc.Bacc("TRN2", target_bir_lowering=False, debug=False)

    # [band, partition, ec*512] per-partition-contiguous input layout
    dIT = nc.dram_tensor("IT", [4, 128, EC * 512], BF16, kind="ExternalInput")
    dXT = nc.dram_tensor("XT", [4, 128, EC * 512], BF16, kind="ExternalInput")
    # mask transposed [k, q], chunked per-ki for fine-grained streaming
    dmT = nc.dram_tensor("maskT", [S, S], mybir.dt.uint8, kind="ExternalInput")
    dW = nc.dram_tensor("Wpack", [E, 3 * H + 1], BF16, kind="ExternalInput")
    dout = nc.dram_tensor("outT", [H + 1, S], F32, kind="ExternalOutput")

    with tile.TileContext(nc) as tc, ExitStack() as ctx:
        singles = ctx.enter_context(tc.tile_pool(name="singles", bufs=1))

        W_all = singles.tile([128, EC, 3 * H + 1], BF16, tag="Wpack")
        nc.sync.dma_start(
            out=W_all, in_=dW.ap().rearrange("(ec p) h -> p ec h", p=128)
        )
        Wq = W_all[:, :, 0:H]
        Wk = W_all[:, :, H:2 * H]
        Wvu = W_all[:, :, 2 * H:3 * H + 1]   # [Wv | u], u = Wk@bq

        IT = singles.tile([128, 4, EC, 512], BF16, tag="IT")
        XT = singles.tile([128, 4, EC, 512], BF16, tag="XT")

        def load_band(dst, src, b):
            nc.sync.dma_start(
                out=dst[:, b, :, :],
                in_=src.ap()[b].rearrange("p (ec s) -> p ec s", ec=EC),
            )

        # HWDGE FIFO order = consumption order; late XT bands feed k-bands
        # whose consumers are mask-delivery-gated anyway, so IT goes first.
        load_band(XT, dXT, 0)
        load_band(IT, dIT, 0)
        load_band(IT, dIT, 1)
        load_band(XT, dXT, 1)
        load_band(IT, dIT, 2)
        load_band(IT, dIT, 3)
        load_band(XT, dXT, 2)
        load_band(XT, dXT, 3)

        # Mask chunks on the SWDGE ring, cast u8 -> bf16 on the wire, at
        # per-ki granularity so delivery paces consumption (~2.5us/chunk).
        # The stream is gated behind ITb0 (via a dummy gpsimd read of it) so
        # the critical head inputs get the fabric exclusively first.
        gate = singles.tile([1, 8], BF16, tag="gate")
        nc.gpsimd.tensor_copy(gate, IT[0:1, 0, 0, 0:8])
        maskT_all = singles.tile([128, SC, S], BF16, tag="maskT")
        for c in range(SC):
            nc.gpsimd.dma_start(
                out=maskT_all[:, c, :],
                in_=dmT.ap()[c * 128:(c + 1) * 128, :],
            )

        # qT/kT mirrored across both partition halves for row-tiled qk
        qTd = singles.tile([128, S], BF16, tag="qTd")
        kTd = singles.tile([128, S], BF16, tag="kTd")
        vD = singles.tile([128, SC, H + 1], BF16, tag="vD")   # [v | d]
        vA = singles.tile([128, SC, H + 1], BF16, tag="vA")   # [v*ed | ed]
        ed = singles.tile([128, SC], BF16, tag="ed")

        # The projection matmuls borrow wp-pool slots (same tag => same
        # 2-bank buffers) so that BOTH per-qh ctx accumulators fit in PSUM:
        # wp 2x2 banks + 2 ctxT x 2 banks = 8.
        psw = ctx.enter_context(tc.tile_pool(name="psw", bufs=2, space="PSUM"))
        psc = ctx.enter_context(tc.tile_pool(name="psc", bufs=1, space="PSUM"))
        ep = ctx.enter_context(tc.tile_pool(name="ep", bufs=3))
        sp = ctx.enter_context(tc.tile_pool(name="sp", bufs=4))
        outp = ctx.enter_context(tc.tile_pool(name="outp", bufs=2))

        def emit_qk_band(W, dstT, srcT, b):
            """One 512-col projection band, duplicated onto both partition
            halves via two concurrent column-tiled matmul chains."""
            pst = psw.tile([128, 2, 512], F32, tag="wp")
            ps = pst[:, 0, :]
            for ei in range(EC):
                rhs = srcT[:, b, ei, :]
                nc.tensor.matmul(
                    ps[0:64, :], lhsT=W[:, ei, :], rhs=rhs,
                    start=(ei == 0), stop=(ei == EC - 1),
                )
                # same data on partitions 64:128 (col groups 2-3); the sim's
                # zero-region checker is partition-base-blind, so skip it
                # (start/stop are per-bank per-partition on hardware).
                nc.tensor.matmul(
                    ps[64:128, :], lhsT=W[:, ei, :], rhs=rhs,
                    start=(ei == 0), stop=(ei == EC - 1),
                    skip_group_check=True,
                )
            nc.vector.tensor_copy(dstT[:, b * 512:(b + 1) * 512], ps)

        def emit_v_group(g):
            """v projection (with d column) for kb chunks 4g..4g+3, then
            ed = exp(d*SCALE) and v_aug = [v*ed | ed] for those chunks."""
            vt = psw.tile([128, 2, 512], F32, tag="wp")
            psv = bass.AP(
                tensor=vt.tensor,
                offset=vt.offset,
                ap=[vt.ap[0], [H + 1, 4], [1, H + 1]],
            )
            for j in range(4):
                kb = 4 * g + j
                b, half = kb // 4, (kb % 4) * 128
                for ei in range(EC):
                    nc.tensor.matmul(
                        psv[:, j, :],
                        lhsT=XT[:, b, ei, half:half + 128],
                        rhs=Wvu[:, ei, :],
                        start=(j == 0 and ei == 0),
                        stop=(j == 3 and ei == EC - 1),
                    )
            sl = slice(4 * g, 4 * g + 4)
            nc.vector.tensor_copy(vD[:, sl, :], psv)
            nc.scalar.activation(ed[:, sl], vD[:, sl, H], AF.Exp, scale=SCALE)
            ed_b = bass.AP(
                tensor=ed.tensor,
                offset=ed.offset + 4 * g * ed.ap[1][0],
                ap=[ed.ap[0], [ed.ap[1][0], 4], [0, H + 1]],
            )
            nc.vector.tensor_tensor(vA[:, sl, :], vD[:, sl, :], ed_b, ALU.mult)
            nc.vector.tensor_copy(vA[:, sl, H], ed[:, sl])

        pend = []

        # Both q-half accumulators live simultaneously so mask chunk p can
        # be consumed for both halves back-to-back (4 iterations = ~4us,
        # matching the ~4us/chunk SWDGE cast-DMA delivery rate).
        ctxA = psc.tile([H + 1, 1024], F32, tag="ctxA")   # q 0:1024
        ctxB = psc.tile([H + 1, 1024], F32, tag="ctxB")   # q 1024:2048

        def emit_iter(qh, p, qq):
            """ki pair (2p, 2p+1) x one q-quarter: row-tiled concurrent qk
            pair, one exp call, DVE mask multiply; ctx matmuls deferred."""
            c0 = qh * 1024 + qq * 512
            wp = psw.tile([128, 2, 512], F32, tag="wp")
            for t in range(2):
                ki = 2 * p + t
                h0 = 64 * t
                nc.tensor.matmul(
                    wp[:, t, :],
                    lhsT=kTd[h0:h0 + 64, ki * 128:(ki + 1) * 128],
                    rhs=qTd[h0:h0 + 64, c0:c0 + 512],
                    start=True,
                    stop=True,
                )
            e = ep.tile([128, 2, 512], BF16, tag="e")
            nc.scalar.activation(e, wp, AF.Exp, scale=SCALE)
            sT = sp.tile([128, 2, 512], BF16, tag="sT")
            nc.vector.tensor_tensor(
                sT, e, maskT_all[:, 2 * p:2 * p + 2, c0:c0 + 512], ALU.mult
            )
            pend.append((sT, qh, p, qq))

        def drain_ctx():
            sT, qh, p, qq = pend.pop(0)
            ctxT = ctxA if qh == 0 else ctxB
            first = (qh == 0 and p == 0) or (qh == 1 and p == 4)
            last = (qh == 0 and p == 7) or (qh == 1 and p == 3)
            for t in range(2):
                ki = 2 * p + t
                nc.tensor.matmul(
                    ctxT[:, qq * 512:(qq + 1) * 512],
                    lhsT=vA[:, ki, :],
                    rhs=sT[:, t, :],
                    start=(first and t == 0),
                    stop=(last and t == 1),
                )

        def step(qh, p, qq):
            emit_iter(qh, p, qq)
            if len(pend) > 1:
                drain_ctx()

        # ---- projections available early ----
        emit_qk_band(Wk, kTd, XT, 0)
        emit_qk_band(Wq, qTd, IT, 0)
        emit_v_group(0)

        # ---- phase A: pairs 0-3, q-half 0 only; later projection bands
        #      slotted into the mask-delivery stall windows ----------------
        step(0, 0, 0)
        emit_qk_band(Wq, qTd, IT, 1)
        step(0, 0, 1)
        step(0, 1, 0)
        step(0, 1, 1)
        emit_v_group(1)
        emit_qk_band(Wk, kTd, XT, 1)
        step(0, 2, 0)
        step(0, 2, 1)
        emit_qk_band(Wq, qTd, IT, 2)
        step(0, 3, 0)
        step(0, 3, 1)
        emit_qk_band(Wq, qTd, IT, 3)
        emit_v_group(2)
        emit_qk_band(Wk, kTd, XT, 2)
        # ---- phase B: pairs 4-7 both q-halves (mask-delivery paced),
        #      braided with the mask-resident qh1 iterations of pairs 0-3
        #      so ACT never idles while a chunk is in flight ----------------
        for p in range(4, 8):
            step(0, p, 0)
            step(0, p, 1)
            step(1, p, 0)
            step(1, p, 1)
            if p == 4:
                emit_v_group(3)
                emit_qk_band(Wk, kTd, XT, 3)
            cp = p - 4
            step(1, cp, 0)
            step(1, cp, 1)
        while pend:
            drain_ctx()
        o = outp.tile([H + 1, 1024], F32, tag="o")
        nc.vector.tensor_copy(o, ctxA)
        nc.sync.dma_start(out=dout.ap()[:, 0:1024], in_=o)
        o2 = outp.tile([H + 1, 1024], F32, tag="o")
        nc.vector.tensor_copy(o2, ctxB)
        nc.sync.dma_start(out=dout.ap()[:, 1024:2048], in_=o2)

    nc.compile()
    return nc


def get_program():
    if "nc" not in _cache:
        _cache["nc"] = _build_program()
    return _cache["nc"]


def _band_pack(mT_bf):
    """[E, S] -> [4, 128, EC*512] with per-partition-contiguous bands."""
    # band b, partition p holds rows {p, 128+p, ..., 640+p} of cols b*512:+512
    return np.ascontiguousarray(
        mT_bf.reshape(EC, 128, 4, 512).transpose(2, 1, 0, 3).reshape(
            4, 128, EC * 512
        )
    )


def make_in_maps(I, x, mask, Wq, bq, Wk, bk, Wv, bv):
    import ml_dtypes

    BF = ml_dtypes.bfloat16
    I = np.asarray(I, dtype=np.float32)
    x = np.asarray(x, dtype=np.float32)
    mask = np.asarray(mask)
    Wq = np.asarray(Wq, dtype=np.float32)
    Wk = np.asarray(Wk, dtype=np.float32)
    Wv = np.asarray(Wv, dtype=np.float32)
    bq = np.asarray(bq, dtype=np.float32)
    u = (Wk @ bq).reshape(E, 1)   # bq-fold column (zero when bq == 0)
    Wpack = np.concatenate([Wq, Wk, Wv, u], axis=1).astype(BF)

    maps = []
    for b in range(B):
        maps.append(
            {
                "IT": _band_pack(I[b].T.astype(BF)),
                "XT": _band_pack(x[b].T.astype(BF)),
                "maskT": np.ascontiguousarray(mask[b].T).astype(np.uint8),
                "Wpack": Wpack,
            }
        )
    return maps


def kernel(I, x, mask, Wq, bq, Wk, bk, Wv, bv):
    nc = get_program()
    in_maps = make_in_maps(I, x, mask, Wq, bq, Wk, bk, Wv, bv)
    res = run_bass_kernel_spmd(nc, in_maps, list(range(N_CORES)))
    bv = np.asarray(bv, dtype=np.float32)
    outs = []
    for b in range(B):
        oT = np.asarray(res.results[b]["outT"], dtype=np.float32)
        outs.append((oT[0:H] / oT[H:H + 1]).T + bv)
    return np.stack(outs, axis=0).astype(np.float32)
